# revision 41
# baseline (speedup 1.0000x reference)
"""Trainium2 Bass kernel for batched global-sum attention (B=8, C=256, N=2048).

Math (per sample b, one NeuronCore each — batch is sharded across 8 cores):
    q = Wq x + bq 1^T ; k = Wk x + bk 1^T ; v = Wv x + bv 1^T        (x: [C,N])
    qk = q^T k ;  attn = v (qk / S) ,  S = sum_b sum(qk_b)

Matmul associativity collapses the two [N,N]-sized products:
    v (q^T k) = (v q^T) k = A x + c 1^T,  A = M Wk,  M = v q^T
    A^T = (Wk^T Wq) U + u1 (x) v0sum + h (x) bv
      with U = G WvT,  G = x x^T,  u1 = Wk^T bq,  v0sum = Wv sx,  sx = x 1,
           h = (Wk^T Wq) sx + N u1
    c   = U^T (Wq^T bk) + (bq.bk) v0sum + s2 bv,  s2 = (Wq^T bk).sx + N bq.bk
    sum(qk_b) = (sum_n q).(sum_m k)

The DEVICE does the O(C^2 N) work entirely in bf16 (fp32 PSUM accumulation):
G = x x^T from a host-pretransposed bf16 xT pack, U = G WvT, AT = QK^T U,
attn0 = AT^T x, c0 = wqb^T U.  All DMA traffic is bf16 (2.25 MB in, 1 MB out
per core) with >=1KB contiguous rows per descriptor, queue-balanced for the
scalar ring's slower first-DMA start.  A PE warmup spin (scratch tile memset
pre-TileContext on GpSimd; output never read) starts right at barrier release
and holds the PE DVFS p-state high while the input stream lands, so the G and
attn matmuls run at the 2.4GHz max p-state.  PSUM->SBUF copies are half-split
across the Vector and Scalar(ACT) engines so each chain stage's gating half
lands early; G consumes xT blocks in DMA arrival order.  The HOST epilogue
applies the exact rank-1/rank-2 bias corrections (all O(C N) or O(C^2),
computed in float64 from sx) and the global 1/S (which couples all samples).
Measured end-to-end rel-err of this dtype plan: 3.7e-3 (harness gate 2e-2).
"""
import sys
sys.path.insert(0, '/opt/trn_rl_repo')
from contextlib import ExitStack

import numpy as np
import ml_dtypes

import concourse.bass as bass
from concourse import bacc
import concourse.mybir as mybir
import concourse.tile as tile
from concourse.bass_utils import run_bass_kernel_spmd

dt = mybir.dt
B, C, N = 8, 256, 2048
NB = N // 128
F32 = dt.float32
BF16 = dt.bfloat16
BFNP = ml_dtypes.bfloat16
Ident = mybir.ActivationFunctionType.Identity

_NC_CACHE = {}


def _build(warmup_mms=9):
    nc = bacc.Bacc("TRN2", target_bir_lowering=False, debug=False)

    xT = nc.declare_dram_parameter("xT", [128, 4096], BF16, isOutput=False)
    xc = nc.declare_dram_parameter("xc", [C, N], BF16, isOutput=False)
    wp = nc.declare_dram_parameter("wp", [C, 516], BF16, isOutput=False)
    attn = nc.declare_dram_parameter("attn", [C, N], BF16, isOutput=True)
    c_out = nc.declare_dram_parameter("c_out", [1, C], F32, isOutput=True)

    # Raw (non-tile) scratch for the warmup spin.  The memset is emitted in
    # the main block ahead of the TileContext, so it runs before the tile
    # prologue barrier — on Vector, which is otherwise idle there (GpSimd is
    # the last engine to reach the rendezvous because of the const-AP
    # memsets, so putting this there would delay the whole kernel).  The
    # barrier itself orders the memset before the PE warmup reads.
    t_warm_raw = nc.alloc_sbuf_tensor("warm_raw", [128, 512], BF16)
    nc.vector.memset(t_warm_raw.ap(), 0.5)

    with tile.TileContext(nc) as tc, ExitStack() as ctx:
        consts = ctx.enter_context(tc.tile_pool(name="consts", bufs=1))
        xpool = ctx.enter_context(tc.tile_pool(name="xpool", bufs=1))
        small = ctx.enter_context(tc.tile_pool(name="small", bufs=1))
        outp = ctx.enter_context(tc.tile_pool(name="outp", bufs=1))
        ps_big = ctx.enter_context(tc.tile_pool(name="ps_big", bufs=4, space="PSUM"))
        ps_g = ctx.enter_context(tc.tile_pool(name="ps_g", bufs=1, space="PSUM"))
        ps_sm = ctx.enter_context(tc.tile_pool(name="ps_sm", bufs=2, space="PSUM"))

        warm_ap = t_warm_raw.ap()
        for wi in range(warmup_mms):
            pw = ps_big.tile([128, 512], F32, name="warm_ps", tag="big")
            nc.tensor.matmul(pw[:], warm_ap[:, 0:128], warm_ap[:, :],
                             start=True, stop=True)

        t_xT = xpool.tile([128, 4096], BF16, name="xTp")
        t_x = [xpool.tile([128, N], BF16, name=f"x{i}") for i in range(2)]
        t_w = [consts.tile([128, 516], BF16, name=f"w_{i}") for i in range(2)]

        # Input DMAs.  xT is front-loaded as 3 chunks per queue, all issued
        # before anything else: with 3 transfers in flight each ring streams
        # near its idle-machine rate instead of dribbling one DMA at a time.
        # sync carries 9 of the 16 xT blocks (the scalar ring starts ~1us
        # later).  w and the x chunks queue behind and reuse DGE semaphores
        # of already-finished xT chunks.
        nc.sync.dma_start(t_xT[:, 0:768], xT[:, 0:768])
        nc.scalar.dma_start(t_xT[:, 2304:2816], xT[:, 2304:2816])
        nc.sync.dma_start(t_xT[:, 768:1536], xT[:, 768:1536])
        nc.scalar.dma_start(t_xT[:, 2816:3328], xT[:, 2816:3328])
        nc.sync.dma_start(t_xT[:, 1536:2304], xT[:, 1536:2304])
        nc.scalar.dma_start(t_xT[:, 3328:4096], xT[:, 3328:4096])
        nc.sync.dma_start(t_x[0][:, 0:1024], xc[0:128, 0:1024])
        nc.scalar.dma_start(t_x[1][:, 0:1024], xc[128:256, 0:1024])
        nc.sync.dma_start(t_w[0][:], wp[0:128, :])
        nc.scalar.dma_start(t_w[1][:], wp[128:256, :])
        nc.sync.dma_start(t_x[0][:, 1024:2048], xc[0:128, 1024:2048])
        nc.scalar.dma_start(t_x[1][:, 1024:2048], xc[128:256, 1024:2048])

        # Dummy activation AFTER the scalar queue's DMA issues: pulls the ACT
        # table load into the streaming window instead of blocking the DGE
        # kick-off or the first real psum copy.
        t_actw = consts.tile([1, 2], F32, name="actw")
        nc.vector.memset(t_actw[:], 0.0)
        t_actw2 = consts.tile([1, 2], F32, name="actw2")
        nc.scalar.activation(t_actw2[:], t_actw[:], Ident)

        t_wvT = [t_w[i][:, 0:256] for i in range(2)]
        t_QK = [t_w[i][:, 256:512] for i in range(2)]
        t_wqb = [t_w[i][:, 512:514] for i in range(2)]

        # G = x x^T accumulated over the 16 transposed n-chunks, consumed in
        # DMA arrival order (first chunk of each queue, then second of each).
        t_G_ps = [ps_g.tile([128, C], F32, name=f"G{ch}", tag=f"G{ch}")
                  for ch in range(2)]
        nb_order = [0, 1, 2, 9, 10, 3, 4, 5, 11, 12, 6, 7, 8, 13, 14, 15]
        for i, nb in enumerate(nb_order):
            xt = t_xT[:, nb * 256:(nb + 1) * 256]
            for ch in range(2):
                nc.tensor.matmul(t_G_ps[ch][:], xt[:, ch * 128:(ch + 1) * 128],
                                 xt, start=(i == 0), stop=(i == NB - 1))
        # Half-split psum->sbuf copies on Vector + GpSimd: the U matmul for
        # output-rows chunk ch only needs column half ch of both G tiles, so
        # it can start after the first halves land.
        t_G = [small.tile([128, C], BF16, name=f"G{ch}") for ch in range(2)]
        for h in range(2):
            hs = slice(h * 128, (h + 1) * 128)
            nc.vector.tensor_copy(t_G[0][:, hs], t_G_ps[0][:, hs])
            nc.scalar.activation(t_G[1][:, hs], t_G_ps[1][:, hs], Ident)

        # U = G WvT, free dim split so the h0 half of each psum region
        # completes (and copies out) before the h1 half is even computed —
        # the downstream AT h0 matmuls gate only on U h0.
        t_U_ps = [ps_sm.tile([128, C], F32, name=f"U_ps{ch}", tag="sm")
                  for ch in range(2)]
        for h in range(2):
            hs = slice(h * 128, (h + 1) * 128)
            for ch in range(2):
                for kt in range(2):
                    nc.tensor.matmul(t_U_ps[ch][:, hs],
                                     t_G[kt][:, ch * 128:(ch + 1) * 128],
                                     t_wvT[kt][:, hs],
                                     start=(kt == 0), stop=(kt == 1))
        # keep-warm filler while the U copies land
        pwf = ps_big.tile([128, 512], F32, name="warm_fill", tag="big")
        nc.tensor.matmul(pwf[:], warm_ap[:, 0:128], warm_ap[:, :],
                         start=True, stop=True)
        t_U = [small.tile([128, C], BF16, name=f"U{ch}") for ch in range(2)]
        for h in range(2):
            hs = slice(h * 128, (h + 1) * 128)
            nc.vector.tensor_copy(t_U[0][:, hs], t_U_ps[0][:, hs])
            nc.scalar.activation(t_U[1][:, hs], t_U_ps[1][:, hs], Ident)

        # AT = QK^T U, same free-dim split: attn's lhsT slices only need the
        # h0 columns of both AT tiles, so the h0 path runs G->U->AT->attn
        # while h1 shadows it.
        t_AT_ps = [ps_sm.tile([128, C], F32, name=f"AT_ps{ch}", tag="sm")
                   for ch in range(2)]
        for h in range(2):
            hs = slice(h * 128, (h + 1) * 128)
            for ch in range(2):
                csl = slice(ch * 128, (ch + 1) * 128)
                for kt in range(2):
                    nc.tensor.matmul(t_AT_ps[ch][:, hs], t_QK[kt][:, csl],
                                     t_U[kt][:, hs],
                                     start=(kt == 0), stop=(kt == 1))
        t_AT = [small.tile([128, C], BF16, name=f"AT{ch}") for ch in range(2)]
        for h in range(2):
            hs = slice(h * 128, (h + 1) * 128)
            nc.vector.tensor_copy(t_AT[0][:, hs], t_AT_ps[0][:, hs])
            nc.scalar.activation(t_AT[1][:, hs], t_AT_ps[1][:, hs], Ident)

        # attn0 = AT^T x, streamed out as bf16: [128,1024] after mc1, then
        # [128,512] per mc so the tail DMA is short.  The tiny c-row matmuls
        # ride between mc0 and mc1 so they stay off the chain's critical path
        # while the c_out DMA still issues well before the output chunks.
        t_out = [outp.tile([128, N], BF16, name=f"out{ch}") for ch in range(2)]
        for mc in range(4):
            msl = slice(mc * 512, (mc + 1) * 512)
            for ch in range(2):
                chsl = slice(ch * 128, (ch + 1) * 128)
                if mc >= 2 and ch == 1:
                    pa = ps_g.tile([128, 512], F32, name=f"attn_ps{mc}_{ch}",
                                   tag=f"G{mc - 2}")
                else:
                    pa = ps_big.tile([128, 512], F32, name=f"attn_ps{mc}_{ch}",
                                     tag="big")
                for kt in range(2):
                    nc.tensor.matmul(pa[:], t_AT[kt][:, chsl], t_x[kt][:, msl],
                                     start=(kt == 0), stop=(kt == 1))
                if ch == 0:
                    nc.vector.tensor_copy(t_out[0][:, msl], pa[:])
                else:
                    nc.scalar.activation(t_out[1][:, msl], pa[:], Ident)
            if mc == 0:
                t_c_ps = ps_sm.tile([2, C], F32, name="c_ps", tag="sm")
                for kt in range(2):
                    nc.tensor.matmul(t_c_ps[:], t_wqb[kt], t_U[kt][:],
                                     start=(kt == 0), stop=(kt == 1))
                t_c0 = small.tile([1, C], F32, name="c0")
                nc.vector.tensor_copy(t_c0[:], t_c_ps[0:1, :])
                # c_out rides the idle GpSimd SWDGE ring so its issue slot
                # doesn't delay the attn output DMAs on the sync ring.
                nc.gpsimd.dma_start(c_out[:], t_c0[:])
            if mc == 0:
                t_c_ps = ps_sm.tile([2, C], F32, name="c_ps", tag="sm")
                for kt in range(2):
                    nc.tensor.matmul(t_c_ps[:], t_wqb[kt], t_U[kt][:],
                                     start=(kt == 0), stop=(kt == 1))
                t_c0 = small.tile([1, C], F32, name="c0")
                nc.vector.tensor_copy(t_c0[:], t_c_ps[0:1, :])
                # c_out rides the idle GpSimd SWDGE ring so its issue slot
                # doesn't delay the attn output DMAs on the sync ring.
                nc.gpsimd.dma_start(c_out[:], t_c0[:])
            elif mc == 1:
                nc.sync.dma_start(attn[0:128, 0:512], t_out[0][:, 0:512])
                nc.scalar.dma_start(attn[128:256, 0:512], t_out[1][:, 0:512])
                nc.sync.dma_start(attn[0:128, msl], t_out[0][:, msl])
                nc.scalar.dma_start(attn[128:256, msl], t_out[1][:, msl])
            elif mc >= 2:
                nc.sync.dma_start(attn[0:128, msl], t_out[0][:, msl])
                nc.scalar.dma_start(attn[128:256, msl], t_out[1][:, msl])

    nc.finalize()
    return nc


def _get_nc():
    if "nc" not in _NC_CACHE:
        _NC_CACHE["nc"] = _build()
    return _NC_CACHE["nc"]


def kernel(x, Wq, bq, Wk, bk, Wv, bv):
    x = np.ascontiguousarray(x, np.float32)
    Wq = np.ascontiguousarray(Wq, np.float32)
    Wk = np.ascontiguousarray(Wk, np.float32)
    Wv = np.ascontiguousarray(Wv, np.float32)
    bq = np.ascontiguousarray(bq, np.float32)
    bk = np.ascontiguousarray(bk, np.float32)
    bv = np.ascontiguousarray(bv, np.float32)
    assert x.shape == (B, C, N), x.shape

    nc = _get_nc()

    Wq64, Wk64, Wv64 = (w.astype(np.float64) for w in (Wq, Wk, Wv))
    bq64, bk64, bv64 = (v.astype(np.float64) for v in (bq, bk, bv))
    QK = Wq64.T @ Wk64                # = (Wk^T Wq)^T
    u1 = Wk64.T @ bq64
    wqb = Wq64.T @ bk64
    bqbk = float(bq64 @ bk64)
    wpk = np.zeros((C, 516), np.float32)
    wpk[:, 0:256] = Wv.T
    wpk[:, 256:512] = QK.astype(np.float32)
    wpk[:, 512] = wqb.astype(np.float32)
    wpk = wpk.astype(BFNP)

    ins = []
    host = []
    for b in range(B):
        x64 = x[b].astype(np.float64)
        sx64 = x64.sum(axis=1)
        qsum = Wq64 @ sx64 + N * bq64
        ksum = Wk64 @ sx64 + N * bk64
        v0sum = Wv64 @ sx64
        h = QK.T @ sx64 + N * u1
        s2 = float(wqb @ sx64) + N * bqbk
        crest = bqbk * v0sum + s2 * bv64
        r1 = u1 @ x64
        r2 = h @ x64
        host.append((float(qsum @ ksum), v0sum, crest, r1, r2))
        xb = x[b].astype(BFNP)
        xTp = np.ascontiguousarray(
            xb.T.reshape(16, 128, 256).transpose(1, 0, 2).reshape(128, 4096))
        ins.append(dict(xc=xb, xT=xTp, wp=wpk))

    res = run_bass_kernel_spmd(nc, ins, list(range(B)))

    S = np.float64(np.sum([hh[0] for hh in host]))
    outs = []
    for b in range(B):
        _, v0sum, crest, r1, r2 = host[b]
        a0 = res.results[b]["attn"].astype(np.float64)
        c_full = res.results[b]["c_out"][0].astype(np.float64) + crest
        full = a0 + np.outer(v0sum, r1) + np.outer(bv64, r2) + c_full[:, None]
        outs.append((full / S).astype(np.float32))
    return np.stack(outs)


if __name__ == "__main__":
    rng = np.random.default_rng(0)
    s = 1.0 / np.sqrt(C)
    inputs = {
        "x": rng.standard_normal((B, C, N), dtype=np.float32),
        "Wq": (rng.standard_normal((C, C)) * s).astype(np.float32),
        "bq": (rng.standard_normal(C) * s).astype(np.float32),
        "Wk": (rng.standard_normal((C, C)) * s).astype(np.float32),
        "bk": (rng.standard_normal(C) * s).astype(np.float32),
        "Wv": (rng.standard_normal((C, C)) * s).astype(np.float32),
        "bv": (rng.standard_normal(C) * s).astype(np.float32),
    }
    out = kernel(**inputs)
    print("kernel output:", out.shape, out.dtype, float(np.abs(out).max()))


# revision 42
# speedup vs baseline: 1.1073x; 1.1073x over previous
"""Trainium2 Bass kernel for batched global-sum attention (B=8, C=256, N=2048).

Math (per sample b, one NeuronCore each — batch is sharded across 8 cores):
    q = Wq x + bq 1^T ; k = Wk x + bk 1^T ; v = Wv x + bv 1^T        (x: [C,N])
    qk = q^T k ;  attn = v (qk / S) ,  S = sum_b sum(qk_b)

Matmul associativity collapses the two [N,N]-sized products:
    v (q^T k) = (v q^T) k = A x + c 1^T,  A = M Wk,  M = v q^T
    A^T = (Wk^T Wq) U + u1 (x) v0sum + h (x) bv
      with U = G WvT,  G = x x^T,  u1 = Wk^T bq,  v0sum = Wv sx,  sx = x 1,
           h = (Wk^T Wq) sx + N u1
    c   = U^T (Wq^T bk) + (bq.bk) v0sum + s2 bv,  s2 = (Wq^T bk).sx + N bq.bk
    sum(qk_b) = (sum_n q).(sum_m k)

The DEVICE does the O(C^2 N) work entirely in bf16 (fp32 PSUM accumulation):
G = x x^T from a host-pretransposed bf16 xT pack, U = G WvT, AT = QK^T U,
attn0 = AT^T x, c0 = wqb^T U.  All DMA traffic is bf16 (2.25 MB in, 1 MB out
per core) with >=1KB contiguous rows per descriptor, queue-balanced for the
scalar ring's slower first-DMA start.  A PE warmup spin (scratch tile memset
pre-TileContext on GpSimd; output never read) starts right at barrier release
and holds the PE DVFS p-state high while the input stream lands, so the G and
attn matmuls run at the 2.4GHz max p-state.  PSUM->SBUF copies are half-split
across the Vector and Scalar(ACT) engines so each chain stage's gating half
lands early; G consumes xT blocks in DMA arrival order.  The HOST epilogue
applies the exact rank-1/rank-2 bias corrections (all O(C N) or O(C^2),
computed in float64 from sx) and the global 1/S (which couples all samples).
Measured end-to-end rel-err of this dtype plan: 3.7e-3 (harness gate 2e-2).
"""
import sys
sys.path.insert(0, '/opt/trn_rl_repo')
from contextlib import ExitStack

import numpy as np
import ml_dtypes

import concourse.bass as bass
from concourse import bacc
import concourse.mybir as mybir
import concourse.tile as tile
from concourse.bass_utils import run_bass_kernel_spmd

dt = mybir.dt
B, C, N = 8, 256, 2048
NB = N // 128
F32 = dt.float32
BF16 = dt.bfloat16
BFNP = ml_dtypes.bfloat16
Ident = mybir.ActivationFunctionType.Identity

_NC_CACHE = {}


def _build(warmup_mms=9):
    nc = bacc.Bacc("TRN2", target_bir_lowering=False, debug=False)

    xT = nc.declare_dram_parameter("xT", [128, 4096], BF16, isOutput=False)
    xc = nc.declare_dram_parameter("xc", [C, N], BF16, isOutput=False)
    wp = nc.declare_dram_parameter("wp", [C, 516], BF16, isOutput=False)
    attn = nc.declare_dram_parameter("attn", [C, N], BF16, isOutput=True)
    c_out = nc.declare_dram_parameter("c_out", [1, C], F32, isOutput=True)

    # Raw (non-tile) scratch for the warmup spin.  The memset is emitted in
    # the main block ahead of the TileContext, so it runs before the tile
    # prologue barrier — on Vector, which is otherwise idle there (GpSimd is
    # the last engine to reach the rendezvous because of the const-AP
    # memsets, so putting this there would delay the whole kernel).  The
    # barrier itself orders the memset before the PE warmup reads.
    t_warm_raw = nc.alloc_sbuf_tensor("warm_raw", [128, 512], BF16)
    nc.vector.memset(t_warm_raw.ap(), 0.5)

    with tile.TileContext(nc) as tc, ExitStack() as ctx:
        consts = ctx.enter_context(tc.tile_pool(name="consts", bufs=1))
        xpool = ctx.enter_context(tc.tile_pool(name="xpool", bufs=1))
        small = ctx.enter_context(tc.tile_pool(name="small", bufs=1))
        outp = ctx.enter_context(tc.tile_pool(name="outp", bufs=1))
        ps_big = ctx.enter_context(tc.tile_pool(name="ps_big", bufs=4, space="PSUM"))
        ps_g = ctx.enter_context(tc.tile_pool(name="ps_g", bufs=1, space="PSUM"))
        ps_sm = ctx.enter_context(tc.tile_pool(name="ps_sm", bufs=2, space="PSUM"))

        warm_ap = t_warm_raw.ap()
        for wi in range(warmup_mms):
            pw = ps_big.tile([128, 512], F32, name="warm_ps", tag="big")
            nc.tensor.matmul(pw[:], warm_ap[:, 0:128], warm_ap[:, :],
                             start=True, stop=True)

        t_xT = xpool.tile([128, 4096], BF16, name="xTp")
        t_x = [xpool.tile([128, N], BF16, name=f"x{i}") for i in range(2)]
        t_w = [consts.tile([128, 516], BF16, name=f"w_{i}") for i in range(2)]

        # Input DMAs.  xT is front-loaded as 3 chunks per queue, all issued
        # before anything else: with 3 transfers in flight each ring streams
        # near its idle-machine rate instead of dribbling one DMA at a time.
        # sync carries 9 of the 16 xT blocks (the scalar ring starts ~1us
        # later).  w and the x chunks queue behind and reuse DGE semaphores
        # of already-finished xT chunks.
        nc.sync.dma_start(t_xT[:, 0:768], xT[:, 0:768])
        nc.scalar.dma_start(t_xT[:, 2304:2816], xT[:, 2304:2816])
        nc.sync.dma_start(t_xT[:, 768:1536], xT[:, 768:1536])
        nc.scalar.dma_start(t_xT[:, 2816:3328], xT[:, 2816:3328])
        nc.sync.dma_start(t_xT[:, 1536:2304], xT[:, 1536:2304])
        nc.scalar.dma_start(t_xT[:, 3328:4096], xT[:, 3328:4096])
        nc.sync.dma_start(t_x[0][:, 0:1024], xc[0:128, 0:1024])
        nc.scalar.dma_start(t_x[1][:, 0:1024], xc[128:256, 0:1024])
        nc.sync.dma_start(t_w[0][:], wp[0:128, :])
        nc.scalar.dma_start(t_w[1][:], wp[128:256, :])
        nc.sync.dma_start(t_x[0][:, 1024:2048], xc[0:128, 1024:2048])
        nc.scalar.dma_start(t_x[1][:, 1024:2048], xc[128:256, 1024:2048])

        # Dummy activation AFTER the scalar queue's DMA issues: pulls the ACT
        # table load into the streaming window instead of blocking the DGE
        # kick-off or the first real psum copy.
        t_actw = consts.tile([1, 2], F32, name="actw")
        nc.vector.memset(t_actw[:], 0.0)
        t_actw2 = consts.tile([1, 2], F32, name="actw2")
        nc.scalar.activation(t_actw2[:], t_actw[:], Ident)

        t_wvT = [t_w[i][:, 0:256] for i in range(2)]
        t_QK = [t_w[i][:, 256:512] for i in range(2)]
        t_wqb = [t_w[i][:, 512:514] for i in range(2)]

        # G = x x^T accumulated over the 16 transposed n-chunks, consumed in
        # DMA arrival order (first chunk of each queue, then second of each).
        t_G_ps = [ps_g.tile([128, C], F32, name=f"G{ch}", tag=f"G{ch}")
                  for ch in range(2)]
        nb_order = [0, 1, 2, 9, 10, 3, 4, 5, 11, 12, 6, 7, 8, 13, 14, 15]
        for i, nb in enumerate(nb_order):
            xt = t_xT[:, nb * 256:(nb + 1) * 256]
            for ch in range(2):
                nc.tensor.matmul(t_G_ps[ch][:], xt[:, ch * 128:(ch + 1) * 128],
                                 xt, start=(i == 0), stop=(i == NB - 1))
        # Half-split psum->sbuf copies on Vector + GpSimd: the U matmul for
        # output-rows chunk ch only needs column half ch of both G tiles, so
        # it can start after the first halves land.
        t_G = [small.tile([128, C], BF16, name=f"G{ch}") for ch in range(2)]
        for h in range(2):
            hs = slice(h * 128, (h + 1) * 128)
            nc.vector.tensor_copy(t_G[0][:, hs], t_G_ps[0][:, hs])
            nc.scalar.activation(t_G[1][:, hs], t_G_ps[1][:, hs], Ident)

        # U = G WvT, free dim split so the h0 half of each psum region
        # completes (and copies out) before the h1 half is even computed —
        # the downstream AT h0 matmuls gate only on U h0.
        t_U_ps = [ps_sm.tile([128, C], F32, name=f"U_ps{ch}", tag="sm")
                  for ch in range(2)]
        for h in range(2):
            hs = slice(h * 128, (h + 1) * 128)
            for ch in range(2):
                for kt in range(2):
                    nc.tensor.matmul(t_U_ps[ch][:, hs],
                                     t_G[kt][:, ch * 128:(ch + 1) * 128],
                                     t_wvT[kt][:, hs],
                                     start=(kt == 0), stop=(kt == 1))
        # keep-warm filler while the U copies land
        pwf = ps_big.tile([128, 512], F32, name="warm_fill", tag="big")
        nc.tensor.matmul(pwf[:], warm_ap[:, 0:128], warm_ap[:, :],
                         start=True, stop=True)
        t_U = [small.tile([128, C], BF16, name=f"U{ch}") for ch in range(2)]
        for h in range(2):
            hs = slice(h * 128, (h + 1) * 128)
            nc.vector.tensor_copy(t_U[0][:, hs], t_U_ps[0][:, hs])
            nc.scalar.activation(t_U[1][:, hs], t_U_ps[1][:, hs], Ident)

        # AT = QK^T U, same free-dim split: attn's lhsT slices only need the
        # h0 columns of both AT tiles, so the h0 path runs G->U->AT->attn
        # while h1 shadows it.
        t_AT_ps = [ps_sm.tile([128, C], F32, name=f"AT_ps{ch}", tag="sm")
                   for ch in range(2)]
        for h in range(2):
            hs = slice(h * 128, (h + 1) * 128)
            for ch in range(2):
                csl = slice(ch * 128, (ch + 1) * 128)
                for kt in range(2):
                    nc.tensor.matmul(t_AT_ps[ch][:, hs], t_QK[kt][:, csl],
                                     t_U[kt][:, hs],
                                     start=(kt == 0), stop=(kt == 1))
        t_AT = [small.tile([128, C], BF16, name=f"AT{ch}") for ch in range(2)]
        for h in range(2):
            hs = slice(h * 128, (h + 1) * 128)
            nc.vector.tensor_copy(t_AT[0][:, hs], t_AT_ps[0][:, hs])
            nc.scalar.activation(t_AT[1][:, hs], t_AT_ps[1][:, hs], Ident)

        # attn0 = AT^T x, streamed out as bf16: [128,1024] after mc1, then
        # [128,512] per mc so the tail DMA is short.  The tiny c-row matmuls
        # ride between mc0 and mc1 so they stay off the chain's critical path
        # while the c_out DMA still issues well before the output chunks.
        t_out = [outp.tile([128, N], BF16, name=f"out{ch}") for ch in range(2)]
        for mc in range(4):
            msl = slice(mc * 512, (mc + 1) * 512)
            for ch in range(2):
                chsl = slice(ch * 128, (ch + 1) * 128)
                pa = ps_big.tile([128, 512], F32, name=f"attn_ps{mc}_{ch}",
                                 tag="big")
                for kt in range(2):
                    nc.tensor.matmul(pa[:], t_AT[kt][:, chsl], t_x[kt][:, msl],
                                     start=(kt == 0), stop=(kt == 1))
                if ch == 0:
                    nc.vector.tensor_copy(t_out[0][:, msl], pa[:])
                else:
                    nc.scalar.activation(t_out[1][:, msl], pa[:], Ident)
            if mc == 0:
                t_c_ps = ps_sm.tile([2, C], F32, name="c_ps", tag="sm")
                for kt in range(2):
                    nc.tensor.matmul(t_c_ps[:], t_wqb[kt], t_U[kt][:],
                                     start=(kt == 0), stop=(kt == 1))
                t_c0 = small.tile([1, C], F32, name="c0")
                nc.vector.tensor_copy(t_c0[:], t_c_ps[0:1, :])
                # c_out rides the idle GpSimd SWDGE ring so its issue slot
                # doesn't delay the attn output DMAs on the sync ring.
                nc.gpsimd.dma_start(c_out[:], t_c0[:])
            if mc == 0:
                t_c_ps = ps_sm.tile([2, C], F32, name="c_ps", tag="sm")
                for kt in range(2):
                    nc.tensor.matmul(t_c_ps[:], t_wqb[kt], t_U[kt][:],
                                     start=(kt == 0), stop=(kt == 1))
                t_c0 = small.tile([1, C], F32, name="c0")
                nc.vector.tensor_copy(t_c0[:], t_c_ps[0:1, :])
                # c_out rides the idle GpSimd SWDGE ring so its issue slot
                # doesn't delay the attn output DMAs on the sync ring.
                nc.gpsimd.dma_start(c_out[:], t_c0[:])
            elif mc == 1:
                nc.sync.dma_start(attn[0:128, 0:512], t_out[0][:, 0:512])
                nc.scalar.dma_start(attn[128:256, 0:512], t_out[1][:, 0:512])
                nc.sync.dma_start(attn[0:128, msl], t_out[0][:, msl])
                nc.scalar.dma_start(attn[128:256, msl], t_out[1][:, msl])
            elif mc >= 2:
                nc.sync.dma_start(attn[0:128, msl], t_out[0][:, msl])
                nc.scalar.dma_start(attn[128:256, msl], t_out[1][:, msl])

    nc.finalize()
    return nc


def _get_nc():
    if "nc" not in _NC_CACHE:
        _NC_CACHE["nc"] = _build()
    return _NC_CACHE["nc"]


def kernel(x, Wq, bq, Wk, bk, Wv, bv):
    x = np.ascontiguousarray(x, np.float32)
    Wq = np.ascontiguousarray(Wq, np.float32)
    Wk = np.ascontiguousarray(Wk, np.float32)
    Wv = np.ascontiguousarray(Wv, np.float32)
    bq = np.ascontiguousarray(bq, np.float32)
    bk = np.ascontiguousarray(bk, np.float32)
    bv = np.ascontiguousarray(bv, np.float32)
    assert x.shape == (B, C, N), x.shape

    nc = _get_nc()

    Wq64, Wk64, Wv64 = (w.astype(np.float64) for w in (Wq, Wk, Wv))
    bq64, bk64, bv64 = (v.astype(np.float64) for v in (bq, bk, bv))
    QK = Wq64.T @ Wk64                # = (Wk^T Wq)^T
    u1 = Wk64.T @ bq64
    wqb = Wq64.T @ bk64
    bqbk = float(bq64 @ bk64)
    wpk = np.zeros((C, 516), np.float32)
    wpk[:, 0:256] = Wv.T
    wpk[:, 256:512] = QK.astype(np.float32)
    wpk[:, 512] = wqb.astype(np.float32)
    wpk = wpk.astype(BFNP)

    ins = []
    host = []
    for b in range(B):
        x64 = x[b].astype(np.float64)
        sx64 = x64.sum(axis=1)
        qsum = Wq64 @ sx64 + N * bq64
        ksum = Wk64 @ sx64 + N * bk64
        v0sum = Wv64 @ sx64
        h = QK.T @ sx64 + N * u1
        s2 = float(wqb @ sx64) + N * bqbk
        crest = bqbk * v0sum + s2 * bv64
        r1 = u1 @ x64
        r2 = h @ x64
        host.append((float(qsum @ ksum), v0sum, crest, r1, r2))
        xb = x[b].astype(BFNP)
        xTp = np.ascontiguousarray(
            xb.T.reshape(16, 128, 256).transpose(1, 0, 2).reshape(128, 4096))
        ins.append(dict(xc=xb, xT=xTp, wp=wpk))

    res = run_bass_kernel_spmd(nc, ins, list(range(B)))

    S = np.float64(np.sum([hh[0] for hh in host]))
    outs = []
    for b in range(B):
        _, v0sum, crest, r1, r2 = host[b]
        a0 = res.results[b]["attn"].astype(np.float64)
        c_full = res.results[b]["c_out"][0].astype(np.float64) + crest
        full = a0 + np.outer(v0sum, r1) + np.outer(bv64, r2) + c_full[:, None]
        outs.append((full / S).astype(np.float32))
    return np.stack(outs)


if __name__ == "__main__":
    rng = np.random.default_rng(0)
    s = 1.0 / np.sqrt(C)
    inputs = {
        "x": rng.standard_normal((B, C, N), dtype=np.float32),
        "Wq": (rng.standard_normal((C, C)) * s).astype(np.float32),
        "bq": (rng.standard_normal(C) * s).astype(np.float32),
        "Wk": (rng.standard_normal((C, C)) * s).astype(np.float32),
        "bk": (rng.standard_normal(C) * s).astype(np.float32),
        "Wv": (rng.standard_normal((C, C)) * s).astype(np.float32),
        "bv": (rng.standard_normal(C) * s).astype(np.float32),
    }
    out = kernel(**inputs)
    print("kernel output:", out.shape, out.dtype, float(np.abs(out).max()))
